# revision 1
# baseline (speedup 1.0000x reference)
"""Trainium2 (trn2) Bass kernel: NT-Xent / SimCLR-style contrastive loss.

Reference computation (N=4096, D=256, T=0.5):
    z      = row-normalize(concat(emb_i, emb_j))          # [2N, D]
    sim    = z @ z.T                                      # [2N, 2N]
    pos_r  = sim[r, (r+N) % 2N]
    denom_r= sum_c exp(sim[r,c]/T) - exp(sim[r,r]/T)
    loss   = mean_r( log(denom_r) - pos_r/T )

Distribution (8 NeuronCores): shard the 2N=8192 row dimension, 1024 rows per
core.  Every core receives the full embedding matrix *rotated* by its row
offset (np.roll on host): its rows are then always rows 0..1023 and the
positive diagonal always sits at column offset +4096, so all 8 cores run one
identical SPMD program.  Row sums are invariant under the column permutation
the rotation induces.  Each core emits two partial scalars
(sum_r log(denom_r), sum_r pos_r); the host combines them into the loss.

Per-core kernel phases:
  A) normalize: per 128-row tile, sum-of-squares (DVE tensor_tensor_reduce),
     1/sqrt via DVE reciprocal + ACT sqrt, scale+cast to bf16, stage z to a
     DRAM scratch, and DMA-xbar-transpose it back as z^T [256, 8192] bf16.
  B) sim row-block: 256 bf16 matmuls [K=128, M=128, N=512] accumulate
     sim chunks [128, 2048] in PSUM; ACT computes exp(2*sim) with the fused
     accum_out row-sum (denominator), DVE extracts the positive diagonal by
     an identity-mask tensor_tensor_reduce.
  C) denom = rowsum - e^2 (the self term exp(sim_rr/T) with sim_rr == 1);
     log via ACT with accum_out, partition-reduce via GPSIMD, DMA out [1,2].
"""

import math

import numpy as np

import concourse.bass as bass
import concourse.mybir as mybir
import concourse.tile as tile
from concourse import bacc
from concourse.bass_utils import run_bass_kernel_spmd

BATCH = 4096
DIM = 256
TEMP = 0.5
N_CORES = 8
TOT = 2 * BATCH          # 8192 total rows
RPC = TOT // N_CORES     # 1024 rows per core
NT = TOT // 128          # 64 row tiles
JB = 8                   # column blocks of 1024 (z^T tiles)
MT = RPC // 128          # 8 m-tiles per core
CHUNK = 2048             # column chunk per PSUM tile
NCHUNK = TOT // CHUNK    # 4
INV_T = 1.0 / TEMP       # 2.0
DIAG_E = math.exp(INV_T)  # exp(sim_rr / T), sim_rr == 1 for unit rows

F32 = mybir.dt.float32
BF16 = mybir.dt.bfloat16

_BUILT = None
LAST_RESULT = None  # test harness reads exec_time_ns from here


def build_nc():
    """Build + compile the single-core SPMD program."""
    AF = mybir.ActivationFunctionType
    OP = mybir.AluOpType
    AX = mybir.AxisListType

    nc = bacc.Bacc("TRN2", target_bir_lowering=False, debug=False)
    embs = nc.dram_tensor("embs", [TOT, DIM], F32, kind="ExternalInput").ap()
    out = nc.dram_tensor("out", [1, 2], F32, kind="ExternalOutput").ap()

    with tile.TileContext(nc) as tc:
        with (
            tc.tile_pool(name="const", bufs=1) as const_pool,
            tc.tile_pool(name="zt", bufs=1) as zt_pool,
            tc.tile_pool(name="stats", bufs=1) as stats_pool,
            tc.tile_pool(name="raw", bufs=5) as raw_pool,
            tc.tile_pool(name="sq", bufs=2) as sq_pool,
            tc.tile_pool(name="zrow", bufs=3) as zrow_pool,
            tc.tile_pool(name="es", bufs=2) as es_pool,
            tc.tile_pool(name="posx", bufs=2) as pos_pool,
            tc.tile_pool(name="zd", bufs=3, space="DRAM") as zd_pool,
            tc.tile_pool(name="mm", bufs=2, space="PSUM") as mm_pool,
        ):
            from concourse.masks import make_identity

            ident = const_pool.tile([128, 128], F32, tag="ident")
            make_identity(nc, ident[:])
            ones = const_pool.tile([128, 1], F32, tag="ones")
            nc.vector.memset(ones[:], 1.0)

            ss = stats_pool.tile([128, NT], F32, tag="ss")
            rss = stats_pool.tile([128, NT], F32, tag="rss")
            inv = stats_pool.tile([128, NT], F32, tag="inv")
            den = stats_pool.tile([128, MT * NCHUNK], F32, tag="den")
            dsum = stats_pool.tile([128, MT], F32, tag="dsum")
            dvals = stats_pool.tile([128, MT], F32, tag="dvals")
            logd = stats_pool.tile([128, MT], F32, tag="logd")
            posc = stats_pool.tile([128, MT], F32, tag="posc")
            fin = stats_pool.tile([128, 2], F32, tag="fin")
            osb = stats_pool.tile([1, 2], F32, tag="osb")


            zt = [
                [
                    zt_pool.tile(
                        [128, CHUNK], BF16, tag=f"zt{kb}_{pr}", name=f"zt{kb}_{pr}"
                    )
                    for pr in range(NCHUNK)
                ]
                for kb in range(2)
            ]

            # per-j-block view: [8 blocks, 128 partitions, 8 subtiles, 256]
            embs_b = embs.rearrange("(j a p) d -> j p a d", p=128, a=8)

            def stats(pr, rsqrt_cols=None):
                """Batched loads (SWDGE ring) + sum-of-squares (+ rsqrt)."""
                for j in (2 * pr, 2 * pr + 1):
                    rawrow = raw_pool.tile(
                        [128, 8 * DIM], F32, tag="raw", name=f"rawrow{j}"
                    )
                    nc.gpsimd.dma_start(out=rawrow[:], in_=embs_b[j])
                    rawrows[j] = rawrow
                    for a in range(8):
                        t = 8 * j + a
                        sq = sq_pool.tile([128, DIM], F32, tag="sq", name="sq")
                        # (raw * 1) * raw elementwise, accum_out row-sum -> sumsq
                        nc.vector.scalar_tensor_tensor(
                            out=sq[:],
                            in0=rawrow[:, a * DIM : (a + 1) * DIM],
                            scalar=1.0,
                            in1=rawrow[:, a * DIM : (a + 1) * DIM],
                            op0=OP.mult,
                            op1=OP.mult,
                            accum_out=ss[:, t : t + 1],
                        )
                if rsqrt_cols is not None:
                    s0, s1 = rsqrt_cols
                    nc.vector.reciprocal(rss[:, s0:s1], ss[:, s0:s1])
                    nc.scalar.activation(inv[:, s0:s1], rss[:, s0:s1], AF.Sqrt)

            def a1(pr):
                """Normalize+cast pair pr, stage to DRAM, xbar-transpose back."""
                zrow = zrow_pool.tile([128, 16 * DIM], BF16, tag="zrow", name="zrow")
                for i, j in enumerate((2 * pr, 2 * pr + 1)):
                    rawrow = rawrows[j]
                    for a in range(8):
                        t = 8 * j + a
                        nc.vector.tensor_scalar_mul(
                            zrow[:, (8 * i + a) * DIM : (8 * i + a + 1) * DIM],
                            rawrow[:, a * DIM : (a + 1) * DIM],
                            inv[:, t : t + 1],
                        )
                zd = zd_pool.tile([CHUNK, DIM], BF16, tag="zd", name="zd")
                nc.sync.dma_start(
                    out=zd[:].rearrange("(a p) d -> p a d", p=128),
                    in_=zrow[:].rearrange("p (a d) -> p a d", a=16),
                )
                for kb in range(2):
                    nc.sync.dma_start(
                        out=zt[kb][pr][:],
                        in_=zd[:, 128 * kb : 128 * (kb + 1)],
                        transpose=True,
                    )

            def bphase(c):
                """One 2048-wide column chunk: matmuls, exp row-sums, positives."""
                for m in range(MT):
                    ps = mm_pool.tile([128, CHUNK], F32, tag="mm", name="mm")
                    for kb in range(2):
                        for h in range(CHUNK // 512):
                            nc.tensor.matmul(
                                ps[:, 512 * h : 512 * (h + 1)],
                                lhsT=zt[kb][0][:, 128 * m : 128 * (m + 1)],
                                rhs=zt[kb][c][:, 512 * h : 512 * (h + 1)],
                                start=(kb == 0),
                                stop=(kb == 1),
                            )
                    es = es_pool.tile([128, CHUNK], BF16, tag="es", name="es")
                    nc.scalar.activation(
                        es[:],
                        ps[:],
                        AF.Exp,
                        scale=INV_T,
                        accum_out=den[:, NCHUNK * m + c : NCHUNK * m + c + 1],
                    )
                    if c == 2:  # chunk holding the positive diagonal (+4096)
                        pos_t = pos_pool.tile([128, 128], F32, tag="posx", name="posx")
                        # identity mask + accum row-sum -> diagonal extract
                        nc.vector.scalar_tensor_tensor(
                            out=pos_t[:],
                            in0=ps[:, 128 * m : 128 * (m + 1)],
                            scalar=1.0,
                            in1=ident[:],
                            op0=OP.mult,
                            op1=OP.mult,
                            accum_out=posc[:, m : m + 1],
                        )

            rawrows = {}
            stats(0, rsqrt_cols=(0, 16))
            a1(0)
            stats(1, rsqrt_cols=(16, 32))
            a1(1)
            bphase(0)
            stats(2)
            stats(3, rsqrt_cols=(32, 64))
            bphase(1)
            a1(2)
            a1(3)
            bphase(2)
            bphase(3)

            # ---------------- Phase C: combine to the two partial scalars ----
            for m in range(MT):
                nc.vector.tensor_reduce(
                    dsum[:, m : m + 1],
                    den[:, NCHUNK * m : NCHUNK * (m + 1)],
                    axis=AX.X,
                    op=OP.add,
                )
            nc.vector.tensor_scalar_add(dvals[:], dsum[:], -DIAG_E)
            nc.scalar.activation(logd[:], dvals[:], AF.Ln, accum_out=fin[:, 0:1])
            nc.vector.tensor_reduce(fin[:, 1:2], posc[:], axis=AX.X, op=OP.add)
            # partition-reduce via ones-vector matmul on PE: [1,2] = ones.T @ fin
            fps = mm_pool.tile([1, 2], F32, tag="mm", name="fin_psum")
            nc.tensor.matmul(fps[:], lhsT=ones[:], rhs=fin[:], start=True, stop=True)
            nc.vector.tensor_copy(osb[:], fps[:])
            nc.sync.dma_start(out=out, in_=osb[:])

    nc.compile()
    return nc


def make_in_maps(emb_i: np.ndarray, emb_j: np.ndarray) -> list[dict]:
    E = np.concatenate(
        [np.asarray(emb_i, np.float32), np.asarray(emb_j, np.float32)], axis=0
    )
    return [
        {"embs": np.ascontiguousarray(np.roll(E, -RPC * k, axis=0))}
        for k in range(N_CORES)
    ]


def combine_partials(partials: list[np.ndarray]) -> np.float32:
    tot = np.zeros(2, dtype=np.float64)
    for p in partials:
        tot += np.asarray(p, np.float64).reshape(2)
    return np.float32((tot[0] - INV_T * tot[1]) / TOT)


def kernel(emb_i: np.ndarray, emb_j: np.ndarray) -> np.float32:
    global _BUILT, LAST_RESULT
    if _BUILT is None:
        _BUILT = build_nc()
    in_maps = make_in_maps(emb_i, emb_j)
    res = run_bass_kernel_spmd(_BUILT, in_maps, list(range(N_CORES)))
    LAST_RESULT = res
    return combine_partials([r["out"] for r in res.results])



# revision 2
# speedup vs baseline: 1.4037x; 1.4037x over previous
"""Trainium2 (trn2) Bass kernel: NT-Xent / SimCLR-style contrastive loss.

Reference computation (N=4096, D=256, T=0.5):
    z      = row-normalize(concat(emb_i, emb_j))          # [2N, D]
    sim    = z @ z.T                                      # [2N, 2N]
    pos_r  = sim[r, (r+N) % 2N]
    denom_r= sum_c exp(sim[r,c]/T) - exp(sim[r,r]/T)
    loss   = mean_r( log(denom_r) - pos_r/T )

Distribution (8 NeuronCores): shard the 2N=8192 row dimension, 1024 rows per
core.  Every core receives the full embedding matrix *rotated* by its row
offset (np.roll on host): its rows are then always rows 0..1023 and the
positive diagonal always sits at column offset +4096, so all 8 cores run one
identical SPMD program.  Row sums are invariant under the column permutation
the rotation induces.  Each core emits two partial scalars
(sum_r log(denom_r), sum_r pos_r); the host combines them into the loss.

Per-core kernel phases:
  A) normalize: per 128-row tile, sum-of-squares (DVE tensor_tensor_reduce),
     1/sqrt via DVE reciprocal + ACT sqrt, scale+cast to bf16, stage z to a
     DRAM scratch, and DMA-xbar-transpose it back as z^T [256, 8192] bf16.
  B) sim row-block: 256 bf16 matmuls [K=128, M=128, N=512] accumulate
     sim chunks [128, 2048] in PSUM; ACT computes exp(2*sim) with the fused
     accum_out row-sum (denominator), DVE extracts the positive diagonal by
     an identity-mask tensor_tensor_reduce.
  C) denom = rowsum - e^2 (the self term exp(sim_rr/T) with sim_rr == 1);
     log via ACT with accum_out, partition-reduce via GPSIMD, DMA out [1,2].
"""

import math

import numpy as np

import concourse.bass as bass
import concourse.mybir as mybir
import concourse.tile as tile
from concourse import bacc
from concourse.bass_utils import run_bass_kernel_spmd

BATCH = 4096
DIM = 256
TEMP = 0.5
N_CORES = 8
TOT = 2 * BATCH          # 8192 total rows
RPC = TOT // N_CORES     # 1024 rows per core
NT = TOT // 128          # 64 row tiles
JB = 8                   # column blocks of 1024 (z^T tiles)
MT = RPC // 128          # 8 m-tiles per core
CHUNK = 2048             # column chunk per PSUM tile
NCHUNK = TOT // CHUNK    # 4
INV_T = 1.0 / TEMP       # 2.0
DIAG_E = math.exp(INV_T)  # exp(sim_rr / T), sim_rr == 1 for unit rows

F32 = mybir.dt.float32
BF16 = mybir.dt.bfloat16

_BUILT = None
LAST_RESULT = None  # test harness reads exec_time_ns from here


def build_nc():
    """Build + compile the single-core SPMD program."""
    AF = mybir.ActivationFunctionType
    OP = mybir.AluOpType
    AX = mybir.AxisListType

    nc = bacc.Bacc("TRN2", target_bir_lowering=False, debug=False)
    embs = nc.dram_tensor("embs", [TOT, DIM], F32, kind="ExternalInput").ap()
    out = nc.dram_tensor("out", [1, 2], F32, kind="ExternalOutput").ap()

    with tile.TileContext(nc) as tc:
        with (
            tc.tile_pool(name="const", bufs=1) as const_pool,
            tc.tile_pool(name="zt", bufs=1) as zt_pool,
            tc.tile_pool(name="stats", bufs=1) as stats_pool,
            tc.tile_pool(name="raw", bufs=5) as raw_pool,
            tc.tile_pool(name="sq", bufs=2) as sq_pool,
            tc.tile_pool(name="zrow", bufs=3) as zrow_pool,
            tc.tile_pool(name="es", bufs=2) as es_pool,
            tc.tile_pool(name="posx", bufs=2) as pos_pool,
            tc.tile_pool(name="zd", bufs=3, space="DRAM") as zd_pool,
            tc.tile_pool(name="mm", bufs=2, space="PSUM") as mm_pool,
        ):
            from concourse.masks import make_identity

            ident = const_pool.tile([128, 128], F32, tag="ident")
            make_identity(nc, ident[:])
            ones = const_pool.tile([128, 1], F32, tag="ones")
            nc.vector.memset(ones[:], 1.0)

            ss = stats_pool.tile([128, NT], F32, tag="ss")
            rss = stats_pool.tile([128, NT], F32, tag="rss")
            inv = stats_pool.tile([128, NT], F32, tag="inv")
            den = stats_pool.tile([128, MT * NCHUNK], F32, tag="den")
            dsum = stats_pool.tile([128, MT], F32, tag="dsum")
            dvals = stats_pool.tile([128, MT], F32, tag="dvals")
            logd = stats_pool.tile([128, MT], F32, tag="logd")
            posc = stats_pool.tile([128, MT], F32, tag="posc")
            fin = stats_pool.tile([128, 2], F32, tag="fin")
            osb = stats_pool.tile([1, 2], F32, tag="osb")


            zt = [
                [
                    zt_pool.tile(
                        [128, CHUNK], BF16, tag=f"zt{kb}_{pr}", name=f"zt{kb}_{pr}"
                    )
                    for pr in range(NCHUNK)
                ]
                for kb in range(2)
            ]

            # per-j-block view: [8 blocks, 128 partitions, 8 subtiles, 256]
            embs_b = embs.rearrange("(j a p) d -> j p a d", p=128, a=8)

            def stats(pr, rsqrt_cols=None):
                """Batched loads (SWDGE ring) + sum-of-squares (+ rsqrt)."""
                for j in (2 * pr, 2 * pr + 1):
                    rawrow = raw_pool.tile(
                        [128, 8 * DIM], F32, tag="raw", name=f"rawrow{j}"
                    )
                    nc.gpsimd.dma_start(out=rawrow[:], in_=embs_b[j])
                    rawrows[j] = rawrow
                    # one wide square + one 8-way segmented reduce per j
                    sq = sq_pool.tile([128, 8 * DIM], F32, tag="sq", name="sq")
                    nc.vector.scalar_tensor_tensor(
                        out=sq[:],
                        in0=rawrow[:],
                        scalar=1.0,
                        in1=rawrow[:],
                        op0=OP.mult,
                        op1=OP.mult,
                    )
                    nc.vector.tensor_reduce(
                        ss[:, 8 * j : 8 * (j + 1)],
                        sq[:].rearrange("p (a d) -> p a d", a=8),
                        axis=AX.X,
                        op=OP.add,
                    )
                if rsqrt_cols is not None:
                    s0, s1 = rsqrt_cols
                    nc.vector.reciprocal(rss[:, s0:s1], ss[:, s0:s1])
                    nc.scalar.activation(inv[:, s0:s1], rss[:, s0:s1], AF.Sqrt)

            def a1(pr):
                """Normalize+cast pair pr, stage to DRAM, xbar-transpose back."""
                zrow = zrow_pool.tile([128, 16 * DIM], BF16, tag="zrow", name="zrow")
                for i, j in enumerate((2 * pr, 2 * pr + 1)):
                    rawrow = rawrows[j]
                    # broadcast inv along each 256-wide subtile via stride-0 AP
                    inv_b = (
                        inv[:, 8 * j : 8 * (j + 1)]
                        .rearrange("p a -> p a ()")
                        .broadcast_to([128, 8, DIM])
                    )
                    nc.vector.scalar_tensor_tensor(
                        out=zrow[:, 8 * i * DIM : (8 * i + 8) * DIM].rearrange(
                            "p (a d) -> p a d", a=8
                        ),
                        in0=rawrow[:].rearrange("p (a d) -> p a d", a=8),
                        scalar=1.0,
                        in1=inv_b,
                        op0=OP.mult,
                        op1=OP.mult,
                    )
                zd = zd_pool.tile([CHUNK, DIM], BF16, tag="zd", name="zd")
                nc.sync.dma_start(
                    out=zd[:].rearrange("(a p) d -> p a d", p=128),
                    in_=zrow[:].rearrange("p (a d) -> p a d", a=16),
                )
                for kb in range(2):
                    nc.sync.dma_start(
                        out=zt[kb][pr][:],
                        in_=zd[:, 128 * kb : 128 * (kb + 1)],
                        transpose=True,
                    )

            def bphase(c):
                """One 2048-wide column chunk: matmuls, exp row-sums, positives."""
                for m in range(MT):
                    ps = mm_pool.tile([128, CHUNK], F32, tag="mm", name="mm")
                    for kb in range(2):
                        for h in range(CHUNK // 512):
                            nc.tensor.matmul(
                                ps[:, 512 * h : 512 * (h + 1)],
                                lhsT=zt[kb][0][:, 128 * m : 128 * (m + 1)],
                                rhs=zt[kb][c][:, 512 * h : 512 * (h + 1)],
                                start=(kb == 0),
                                stop=(kb == 1),
                            )
                    es = es_pool.tile([128, CHUNK], BF16, tag="es", name="es")
                    nc.scalar.activation(
                        es[:],
                        ps[:],
                        AF.Exp,
                        scale=INV_T,
                        accum_out=den[:, NCHUNK * m + c : NCHUNK * m + c + 1],
                    )
                    if c == 2:  # chunk holding the positive diagonal (+4096)
                        pos_t = pos_pool.tile([128, 128], F32, tag="posx", name="posx")
                        # identity mask + accum row-sum -> diagonal extract
                        nc.vector.scalar_tensor_tensor(
                            out=pos_t[:],
                            in0=ps[:, 128 * m : 128 * (m + 1)],
                            scalar=1.0,
                            in1=ident[:],
                            op0=OP.mult,
                            op1=OP.mult,
                            accum_out=posc[:, m : m + 1],
                        )

            rawrows = {}
            stats(0, rsqrt_cols=(0, 16))
            a1(0)
            stats(1, rsqrt_cols=(16, 32))
            a1(1)
            bphase(0)
            stats(2)
            stats(3, rsqrt_cols=(32, 64))
            bphase(1)
            a1(2)
            a1(3)
            bphase(2)
            bphase(3)

            # ---------------- Phase C: combine to the two partial scalars ----
            for m in range(MT):
                nc.vector.tensor_reduce(
                    dsum[:, m : m + 1],
                    den[:, NCHUNK * m : NCHUNK * (m + 1)],
                    axis=AX.X,
                    op=OP.add,
                )
            nc.vector.tensor_scalar_add(dvals[:], dsum[:], -DIAG_E)
            nc.scalar.activation(logd[:], dvals[:], AF.Ln, accum_out=fin[:, 0:1])
            nc.vector.tensor_reduce(fin[:, 1:2], posc[:], axis=AX.X, op=OP.add)
            # partition-reduce via ones-vector matmul on PE: [1,2] = ones.T @ fin
            fps = mm_pool.tile([1, 2], F32, tag="mm", name="fin_psum")
            nc.tensor.matmul(fps[:], lhsT=ones[:], rhs=fin[:], start=True, stop=True)
            nc.vector.tensor_copy(osb[:], fps[:])
            nc.sync.dma_start(out=out, in_=osb[:])

    nc.compile()
    return nc


def make_in_maps(emb_i: np.ndarray, emb_j: np.ndarray) -> list[dict]:
    E = np.concatenate(
        [np.asarray(emb_i, np.float32), np.asarray(emb_j, np.float32)], axis=0
    )
    return [
        {"embs": np.ascontiguousarray(np.roll(E, -RPC * k, axis=0))}
        for k in range(N_CORES)
    ]


def combine_partials(partials: list[np.ndarray]) -> np.float32:
    tot = np.zeros(2, dtype=np.float64)
    for p in partials:
        tot += np.asarray(p, np.float64).reshape(2)
    return np.float32((tot[0] - INV_T * tot[1]) / TOT)


def kernel(emb_i: np.ndarray, emb_j: np.ndarray) -> np.float32:
    global _BUILT, LAST_RESULT
    if _BUILT is None:
        _BUILT = build_nc()
    in_maps = make_in_maps(emb_i, emb_j)
    res = run_bass_kernel_spmd(_BUILT, in_maps, list(range(N_CORES)))
    LAST_RESULT = res
    return combine_partials([r["out"] for r in res.results])

